# revision 2
# baseline (speedup 1.0000x reference)
"""Trainium2 Bass kernel for LinearTransformerExpert.

Reference computation (per token n, 16 heads, head_dim 128, prefix len 8):
    q = x @ Wq.T ;  k = prefix @ Wk.T ;  v = prefix @ Wv.T
    scores[n,h,p] = q[n,h,:] . k[n,p,h,:] / sqrt(D)
    attn = softmax_p(scores);  out[n,h,:] = sum_p attn * v
    result = (out @ Wo.T) * sigmoid(x @ Wg.T + bg)

Strategy: data-parallel over the 8192 tokens across 8 NeuronCores (1024
tokens each), no collectives. All matmuls run in bf16 on the PE with the
activations as the stationary operand, so every projection lands in PSUM
token-major ([token, out_channel]) — exactly the layout the attention
(computed on DVE/ACT, hidden under the PE) and the final output DMA want.
The only transposes are the 16x8 [128,128] PE transposes of the attention
output needed to feed the Wo projection.
"""

import math
import os

import numpy as np
import ml_dtypes

import concourse.bass as bass
import concourse.bacc as bacc
import concourse.mybir as mybir
from concourse import tile
from concourse.masks import make_identity

BF16 = mybir.dt.bfloat16
F32 = mybir.dt.float32
NPBF16 = ml_dtypes.bfloat16

# problem shape (hardcoded; kernel.py must be self-contained)
N, P, C, H = 8192, 8, 2048, 16
D = C // H                    # 128
NCORES = 8
NTOK = N // NCORES            # 1024 tokens per core
NT = NTOK // 128              # 8 token-tiles per core
CT = C // 128                 # 16 contraction tiles
OH = 2                        # output-channel halves (1024 each)
HPH = H // OH                 # heads per half = 8
SCALE = 1.0 / math.sqrt(D)


def _bcast(ap, n):
    """Append a 0-stride broadcast dim of size n to an AP."""
    return bass.AP(ap.tensor, ap.offset, list(ap.ap) + [[0, n]])


def build_nc(nt=NT):
    """Build the per-core SPMD program for `nt` token-tiles (nt*128 tokens)."""
    ntok = nt * 128
    nc = bacc.Bacc("TRN2", target_bir_lowering=False, debug=False,
                   num_devices=NCORES)

    xT = nc.dram_tensor("xT", [C, ntok], BF16, kind="ExternalInput")
    pT = nc.dram_tensor("pT", [C, P, ntok], BF16, kind="ExternalInput")
    wq = nc.dram_tensor("wq", [C, C], BF16, kind="ExternalInput")
    wk = nc.dram_tensor("wk", [C, C], BF16, kind="ExternalInput")
    wv = nc.dram_tensor("wv", [C, C], BF16, kind="ExternalInput")
    wo = nc.dram_tensor("wo", [C, C], BF16, kind="ExternalInput")
    wg = nc.dram_tensor("wg", [C, C], BF16, kind="ExternalInput")
    bg = nc.dram_tensor("bg", [1, C], BF16, kind="ExternalInput")
    out = nc.dram_tensor("out", [ntok, C], F32, kind="ExternalOutput")

    q_spill = nc.dram_tensor("q_spill", [nt, 128, C], BF16)
    o_spill = nc.dram_tensor("o_spill", [nt, 128, C], BF16)

    def wview(w):  # [C, C] dram -> [128, CT, C] (partition, ktile, outch)
        return w[:].rearrange("(t p) o -> p t o", p=128)

    def xview(j):  # stationary block for token-tile j: [128, CT, 128]
        return xT[:, j * 128:(j + 1) * 128].rearrange("(t p) n -> p t n", p=128)

    def pview(j, p):
        return pT[:, p, j * 128:(j + 1) * 128].rearrange("(t p) n -> p t n", p=128)

    with tile.TileContext(nc) as tc:
        # ---------------- Phase 1: q = x @ Wq.T (scaled) ----------------
        with tc.tile_pool(name="p1w", bufs=1) as p1w, \
             tc.tile_pool(name="p1x", bufs=2) as p1x, \
             tc.tile_pool(name="p1o", bufs=2) as p1o, \
             tc.tile_pool(name="p1ps", bufs=2, space="PSUM") as p1ps:
            wq_sb = p1w.tile([128, CT * C], BF16, tag="wq")
            wq_v = wq_sb[:].rearrange("p (t o) -> p t o", t=CT)
            nc.sync.dma_start(out=wq_v, in_=wview(wq))
            for j in range(nt):
                xb = p1x.tile([128, CT * 128], BF16, tag="xb")
                xb_v = xb[:].rearrange("p (t n) -> p t n", t=CT)
                nc.sync.dma_start(out=xb_v, in_=xview(j))
                qp = p1ps.tile([128, C], F32, tag="qp")
                for t in range(CT):
                    for c4 in range(C // 512):
                        nc.tensor.matmul(
                            qp[:, c4 * 512:(c4 + 1) * 512],
                            xb_v[:, t, :],
                            wq_v[:, t, c4 * 512:(c4 + 1) * 512],
                            start=(t == 0), stop=(t == CT - 1))
                qb = p1o.tile([128, C], BF16, tag="qb")
                nc.scalar.mul(qb[:], qp[:], SCALE)
                nc.sync.dma_start(out=q_spill[j], in_=qb[:])

        # ------- Phase 2: k/v projection + attention, fused per tile -------
        with tc.tile_pool(name="p2w", bufs=1) as p2w, \
             tc.tile_pool(name="p2pf", bufs=2) as p2pf, \
             tc.tile_pool(name="p2q", bufs=2) as p2q, \
             tc.tile_pool(name="p2kv", bufs=2) as p2kv, \
             tc.tile_pool(name="p2acc", bufs=1) as p2acc, \
             tc.tile_pool(name="p2sc", bufs=3) as p2sc, \
             tc.tile_pool(name="p2sm", bufs=3) as p2sm, \
             tc.tile_pool(name="p2out", bufs=2) as p2out, \
             tc.tile_pool(name="p2psk", bufs=2, space="PSUM") as p2psk, \
             tc.tile_pool(name="p2psv", bufs=2, space="PSUM") as p2psv:
            wk_sb = p2w.tile([128, CT * C], BF16, tag="wk")
            wv_sb = p2w.tile([128, CT * C], BF16, tag="wv")
            wk_v = wk_sb[:].rearrange("p (t o) -> p t o", t=CT)
            wv_v = wv_sb[:].rearrange("p (t o) -> p t o", t=CT)
            nc.sync.dma_start(out=wk_v, in_=wview(wk))
            nc.sync.dma_start(out=wv_v, in_=wview(wv))

            for j in range(nt):
                qb = p2q.tile([128, C], BF16, tag="q")
                nc.sync.dma_start(out=qb[:], in_=q_spill[j])
                O = p2acc.tile([128, C], F32, tag="O")
                s_den = p2sm.tile([128, H], F32, tag="sden")
                for p in range(P):
                    pf = p2pf.tile([128, CT * 128], BF16, tag="pf")
                    pf_v = pf[:].rearrange("p (t n) -> p t n", t=CT)
                    nc.sync.dma_start(out=pf_v, in_=pview(j, p))
                    for hf in range(OH):
                        o0 = hf * (C // OH)
                        kp = p2psk.tile([128, C // OH], F32, tag="kp")
                        vp = p2psv.tile([128, C // OH], F32, tag="vp")
                        for t in range(CT):
                            for c2 in range(C // OH // 512):
                                sl = slice(c2 * 512, (c2 + 1) * 512)
                                wsl = slice(o0 + c2 * 512, o0 + (c2 + 1) * 512)
                                nc.tensor.matmul(
                                    kp[:, sl], pf_v[:, t, :], wk_v[:, t, wsl],
                                    start=(t == 0), stop=(t == CT - 1))
                                nc.tensor.matmul(
                                    vp[:, sl], pf_v[:, t, :], wv_v[:, t, wsl],
                                    start=(t == 0), stop=(t == CT - 1))
                        kb = p2kv.tile([128, C // OH], BF16, tag="kb")
                        vb = p2kv.tile([128, C // OH], BF16, tag="vb")
                        nc.scalar.copy(kb[:], kp[:])
                        nc.vector.tensor_copy(vb[:], vp[:])
                        # scores for heads of this half: [128, HPH]
                        prod = p2sc.tile([128, C // OH], F32, tag="prod")
                        nc.vector.tensor_mul(prod[:], qb[:, o0:o0 + C // OH], kb[:])
                        sc = p2sm.tile([128, HPH], F32, tag="sc")
                        nc.vector.tensor_reduce(
                            sc[:], prod[:].rearrange("p (h d) -> p h d", d=D),
                            mybir.AxisListType.X, mybir.AluOpType.add)
                        ee = p2sm.tile([128, HPH], F32, tag="ee")
                        nc.scalar.activation(ee[:], sc[:],
                                             mybir.ActivationFunctionType.Exp)
                        s_sl = s_den[:, hf * HPH:(hf + 1) * HPH]
                        if p == 0:
                            nc.vector.tensor_copy(s_sl, ee[:])
                        else:
                            nc.vector.tensor_add(s_sl, s_sl, ee[:])
                        # O[:, half] (+)= ee_bcast * v
                        O_v = O[:, o0:o0 + C // OH].rearrange(
                            "p (h d) -> p h d", d=D)
                        v_v = vb[:].rearrange("p (h d) -> p h d", d=D)
                        e_b = _bcast(ee[:], D)
                        if p == 0:
                            nc.vector.tensor_tensor(O_v, v_v, e_b,
                                                    mybir.AluOpType.mult)
                        else:
                            tmp = p2sc.tile([128, C // OH], F32, tag="prod")
                            tmp_v = tmp[:].rearrange("p (h d) -> p h d", d=D)
                            nc.vector.tensor_tensor(tmp_v, v_v, e_b,
                                                    mybir.AluOpType.mult)
                            nc.vector.tensor_add(
                                O[:, o0:o0 + C // OH], O[:, o0:o0 + C // OH],
                                tmp[:])
                # normalize and spill attention output
                s_inv = p2sm.tile([128, H], F32, tag="sinv")
                nc.vector.reciprocal(s_inv[:], s_den[:])
                ob = p2out.tile([128, C], BF16, tag="ob")
                nc.vector.tensor_tensor(
                    ob[:].rearrange("p (h d) -> p h d", d=D),
                    O[:].rearrange("p (h d) -> p h d", d=D),
                    _bcast(s_inv[:], D), mybir.AluOpType.mult)
                nc.sync.dma_start(out=o_spill[j], in_=ob[:])

        # ---- Phase 3a: transpose attention outputs (outT = o_spill^T) ----
        with tc.tile_pool(name="p3t", bufs=1) as p3t:
            oT = []
            with tc.tile_pool(name="p3i", bufs=1) as p3i, \
                 tc.tile_pool(name="p3ob", bufs=2) as p3ob, \
                 tc.tile_pool(name="p3ps", bufs=2, space="PSUM") as p3ps:
                ident = p3i.tile([128, 128], BF16, tag="ident")
                make_identity(nc, ident[:])
                for j in range(nt):
                    oT_j = p3t.tile([128, CT * 128], BF16, tag=f"oT{j}")
                    oT.append(oT_j)
                    ob = p3ob.tile([128, C], BF16, tag="ob3")
                    nc.sync.dma_start(out=ob[:], in_=o_spill[j])
                    for t in range(CT):
                        tp = p3ps.tile([128, 128], BF16, tag="tp")
                        nc.tensor.transpose(tp[:], ob[:, t * 128:(t + 1) * 128],
                                            ident[:])
                        nc.scalar.copy(oT_j[:, t * 128:(t + 1) * 128], tp[:])

            # ---- Phase 3b: result = (out @ Wo.T) * sigmoid(x @ Wg.T + bg) ----
            with tc.tile_pool(name="p3w", bufs=1) as p3w, \
                 tc.tile_pool(name="p3x", bufs=2) as p3x, \
                 tc.tile_pool(name="p3g", bufs=2) as p3g, \
                 tc.tile_pool(name="p3f", bufs=2) as p3f, \
                 tc.tile_pool(name="p3c", bufs=1) as p3c, \
                 tc.tile_pool(name="p3psf", bufs=2, space="PSUM") as p3psf, \
                 tc.tile_pool(name="p3psz", bufs=2, space="PSUM") as p3psz:
                ones_sb = p3c.tile([1, 128], BF16, tag="ones")
                nc.vector.memset(ones_sb[:], 1.0)
                bg_sb = p3c.tile([1, C], BF16, tag="bgs")
                nc.sync.dma_start(out=bg_sb[:], in_=bg[:])
                for hf in range(OH):
                    o0 = hf * (C // OH)
                    wo_sb = p3w.tile([128, CT * (C // OH)], BF16, tag="wo")
                    wg_sb = p3w.tile([128, CT * (C // OH)], BF16, tag="wg")
                    wo_v = wo_sb[:].rearrange("p (t o) -> p t o", t=CT)
                    wg_v = wg_sb[:].rearrange("p (t o) -> p t o", t=CT)
                    nc.sync.dma_start(out=wo_v, in_=wview(wo)[:, :, o0:o0 + C // OH])
                    nc.sync.dma_start(out=wg_v, in_=wview(wg)[:, :, o0:o0 + C // OH])
                    for j in range(nt):
                        xb = p3x.tile([128, CT * 128], BF16, tag="xb3")
                        xb_v = xb[:].rearrange("p (t n) -> p t n", t=CT)
                        nc.sync.dma_start(out=xb_v, in_=xview(j))
                        oT_v = oT[j][:].rearrange("p (t n) -> p t n", t=CT)
                        fp = p3psf.tile([128, C // OH], F32, tag="fp")
                        zp = p3psz.tile([128, C // OH], F32, tag="zp")
                        for t in range(CT):
                            for c2 in range(C // OH // 512):
                                sl = slice(c2 * 512, (c2 + 1) * 512)
                                wsl = slice(c2 * 512, (c2 + 1) * 512)
                                nc.tensor.matmul(
                                    fp[:, sl], oT_v[:, t, :], wo_v[:, t, wsl],
                                    start=(t == 0), stop=(t == CT - 1))
                                nc.tensor.matmul(
                                    zp[:, sl], xb_v[:, t, :], wg_v[:, t, wsl],
                                    start=(t == 0), stop=False)
                        for c2 in range(C // OH // 512):
                            sl = slice(c2 * 512, (c2 + 1) * 512)
                            nc.tensor.matmul(
                                zp[:, sl], ones_sb[:],
                                bg_sb[:, o0 + c2 * 512:o0 + (c2 + 1) * 512],
                                start=False, stop=True)
                        gb = p3g.tile([128, C // OH], F32, tag="gb")
                        nc.scalar.activation(gb[:], zp[:],
                                             mybir.ActivationFunctionType.Sigmoid)
                        fb = p3f.tile([128, C // OH], F32, tag="fb")
                        nc.vector.tensor_mul(fb[:], fp[:], gb[:])
                        nc.sync.dma_start(
                            out=out[j * 128:(j + 1) * 128, o0:o0 + C // OH],
                            in_=fb[:])

    nc.compile()
    return nc


_NC_CACHE = {}


def _get_nc(nt=NT):
    if nt not in _NC_CACHE:
        _NC_CACHE[nt] = build_nc(nt)
    return _NC_CACHE[nt]


def prep_core_inputs(x, prefix, Wq, Wk, Wv, Wo, Wg, bg):
    """Shard + lay out host inputs for the 8 cores."""
    wqt = np.ascontiguousarray(Wq.T).astype(NPBF16)
    wkt = np.ascontiguousarray(Wk.T).astype(NPBF16)
    wvt = np.ascontiguousarray(Wv.T).astype(NPBF16)
    wot = np.ascontiguousarray(Wo.T).astype(NPBF16)
    wgt = np.ascontiguousarray(Wg.T).astype(NPBF16)
    bgb = np.ascontiguousarray(bg.reshape(1, C)).astype(NPBF16)
    in_maps = []
    for c in range(NCORES):
        sl = slice(c * NTOK, (c + 1) * NTOK)
        xT = np.ascontiguousarray(x[sl].T).astype(NPBF16)           # [C, NTOK]
        pT = np.ascontiguousarray(prefix[sl].transpose(2, 1, 0)).astype(NPBF16)
        in_maps.append({"xT": xT, "pT": pT, "wq": wqt, "wk": wkt,
                        "wv": wvt, "wo": wot, "wg": wgt, "bg": bgb})
    return in_maps


def kernel(x, prefix, Wq, Wk, Wv, Wo, Wg, bg):
    from concourse.bass_utils import run_bass_kernel_spmd
    x = np.asarray(x, dtype=np.float32)
    prefix = np.asarray(prefix, dtype=np.float32)
    in_maps = prep_core_inputs(x, prefix, np.asarray(Wq), np.asarray(Wk),
                               np.asarray(Wv), np.asarray(Wo), np.asarray(Wg),
                               np.asarray(bg))
    nc = _get_nc()
    res = run_bass_kernel_spmd(nc, in_maps, core_ids=list(range(NCORES)))
    return np.concatenate([res.results[c]["out"] for c in range(NCORES)], axis=0)
